# revision 41
# baseline (speedup 1.0000x reference)
"""AvgPool2d(64x64, stride 1, auto_pad-replicate) on TRN2, 8 NeuronCores.

Reference computes, per (n, c) plane X [256, 256]:
    inner = box_sum_64x64(X) / 4096            # [193, 193]
    out[io, jo] = inner[clamp(io-31, 0, 192), clamp(jo-31, 0, 192)]

Two matmul stages (inner = Bv^T @ X @ Bw with banded 0/1 matrices):
    stage A: fp8e4 DoubleRow matmul(lhsT=X [h=2 k-tiles x 128, w-chunk],
             rhs=Bv [h, io]) -> Y^T chunk [w, io]; ONE matmul per
             (plane, w-chunk): the full 256-row h contraction happens in
             one pass, so the 63-column band overlap of two separate
             128-row chunks disappears.
    stage B: bf16.  io-chunk 0 ([io 0:128] x jo) as 3 banded matmuls
             streaming the 128/63/65-col nonzero jo support per w-chunk;
             io-chunk 1 transposed ([jo] x [io 128:193], 65 cols
             streamed) so its PSUM footprint is 130 cols instead of 193
             -- out-of-support jo columns of each w-chunk are all-zero
             weight columns, which cost nothing.

The bottleneck is PSUM evacuation: only vector (DVE) and scalar (ACT)
can read PSUM (GPSIMD cannot, and DMA from PSUM is disallowed), and
fp32 PSUM reads run at ~1 elem/partition/cycle with no fast mode.  Per
2-plane group that is y [128,2,386] (one CAST, ~955ns DVE) plus
o [128,2,323] (one ACTIVATE, ~806ns ACT).  All y-evacs go to vector
(~61us) and all o-evacs to scalar (~51us): migrating y ops to scalar
to balance the totals measures WORSE, because a migrated y queues
behind scalar's o backlog and stalls the whole PSUM rotation.  PSUM is
exactly full: 2 double-buffered 2-bank tiles each for y and o.

Other structure:
  * Input is fp8 E4M3 (error-diffused along w, rel err ~5e-3 total)
    packed partition-major [r, plane, k, w], h = 128k + r, matching the
    DoubleRow [K, 2, M] k-tile-major operand layout.
  * Band consts ride scalar's own HW DMA queue in parallel with the
    first x slivers on the sync queue; PE warms up on dummy matmuls
    until they land.
  * Single merged output tensor outm [128, plane, 323] = [c0 | A | B]
    regions; each flush is one dense ~10KB-per-partition DMA from the
    otherwise idle gpsimd's queue, in half-tile (8-plane) chunks; the
    final tile drains in 2-4 plane slices alternating onto the by-then
    idle sync input queue.

Sharding: pure data parallel, batch dim 16 -> 2 per core, 128 (n,c)
planes per core. No collectives.
"""

import ml_dtypes
import numpy as np

import concourse.bass as bass
import concourse.tile as tile
from concourse import mybir
from concourse.bass_utils import run_bass_kernel_spmd


N_CORES = 8
N, C, H, W = 16, 64, 256, 256
KPOOL = 64
PLANES_PER_CORE = (N // N_CORES) * C  # 128
OUT_I = H - KPOOL + 1  # 193 distinct output rows/cols
M2 = OUT_I - 128  # 65, second io chunk
PAD_LO = (H - OUT_I) // 2  # 31
PAD_HI = H - OUT_I - PAD_LO  # 32

X_DT = mybir.dt.float8e4
X_NP = ml_dtypes.float8_e4m3
MM_DT = mybir.dt.bfloat16
MM_NP = ml_dtypes.bfloat16
OUT_DT = mybir.dt.bfloat16

BATCH = 16  # planes per input DMA batch
GROUPS = PLANES_PER_CORE // 2  # 64 2-plane evacuation groups
PIPE = 4  # software-pipeline distance between stage A and B, in groups
FLUSH = 8  # groups per output DMA flush tile (16 planes)
N_WARM = 24  # PE warmup matmuls while the first input DMA lands


def _evac_assignment():
    """Fixed evacuation lanes: ALL y -> vector, ALL o -> scalar.  Migrating
    y ops to scalar balances the engine totals on paper (DVE 61.1us vs ACT
    51.6us) but each migrated y queues behind scalar's in-order o backlog,
    delaying that group's stage B and stalling the whole PSUM rotation for
    ~1.5us -- measured worse than the imbalance.  End-of-pipeline lane
    swaps (first y ops to scalar / last o to vector) were also measured:
    the latency chain just re-locks elsewhere, net zero.  GPSIMD
    instructions cannot access PSUM (BIR verifier rule), so it only
    pushes output descriptors."""
    seq = []
    for g in range(GROUPS):
        seq.append("v")  # y-evac of group g
        seq.append("s")  # o-evac of group g
    return seq


_EVAC_ENG = _evac_assignment()


def _band(n: int, k: int, scale: float) -> np.ndarray:
    """B[i, o] = scale if o <= i < o + k else 0;  [n, n-k+1]."""
    m = n - k + 1
    b = np.zeros((n, m), dtype=np.float32)
    for o in range(m):
        b[o : o + k, o] = scale
    return b


def _diffuse_fp8(x: np.ndarray) -> np.ndarray:
    """Quantize to E4M3 with error feedback along the last axis.

    Box sums of the quantized tensor then differ from exact by only the
    boundary carries, ~5x less error than round-to-nearest.
    """
    out = np.empty(x.shape, dtype=X_NP)
    c = np.zeros(x.shape[:-1], dtype=np.float32)
    for j in range(x.shape[-1]):
        v = x[..., j] + c
        q = v.astype(X_NP)
        c = v - np.asarray(q, dtype=np.float32)
        out[..., j] = q
    return out


def _split_multiwaits(nc: bass.Bass) -> None:
    """Walrus codegen allows a single sync-wait slot per compute instruction.

    Tile's semaphore assignment can emit several; hoist the extras onto
    standalone NOPs (which lower to pure sequencer waits) in front of the
    instruction, on the same engine, preserving order and semantics.
    """
    f = nc.m.functions[0]
    for block in f.blocks:
        out = []
        for inst in block.instructions:
            si = inst.sync_info
            if si is not None and len(si.on_wait) > 1:
                waits = list(si.on_wait)
                for w in waits[:-1]:
                    nop = mybir.InstNoOp(name=f"WS-{nc.next_id()}", ins=[], outs=[])
                    nop.engine = inst.engine
                    nop.sync_info = mybir.SyncInfo(on_wait=[w], on_update=[])
                    out.append(nop)
                inst.sync_info = mybir.SyncInfo(
                    on_wait=[waits[-1]], on_update=list(si.on_update)
                )
            out.append(inst)
        block.instructions = out


def _build(split_waits: bool = True) -> bass.Bass:
    nc = bass.Bass()
    # partition-major layouts: x [r, plane, k, w]; h = k*128 + r
    x_ext = nc.declare_dram_parameter(
        "x", [128, PLANES_PER_CORE, 2, W], X_DT, isOutput=False
    )
    bv_ext = nc.declare_dram_parameter("bv", [H, OUT_I], X_DT, isOutput=False)
    bw_ext = nc.declare_dram_parameter("bw", [W, OUT_I], MM_DT, isOutput=False)
    # Single merged output: per plane 323 columns = [c0: inner[io 0:128, jo]
    # (193 cols, io on partitions)] [A: inner[io 128:193, jo 0:128]^T (65
    # cols, jo on partitions)] [B: inner[io 128:193, jo 128:193]^T (65 cols,
    # jo-128 on partitions 0:65)].  The transposed c1 regions keep the evac
    # free-size at 323/plane instead of 386; partitions 65..127 of region B
    # ship ~0.5 MB of dead bytes, cheaper than a second DMA stream.
    outm_ext = nc.declare_dram_parameter(
        "outm", [128, PLANES_PER_CORE, OUT_I + 2 * M2], OUT_DT, isOutput=True
    )

    n_batches = PLANES_PER_CORE // BATCH

    def evac_copy(eng: str, out, in_):
        if eng == "v":
            nc.vector.tensor_copy(out, in_)
        elif eng == "s":
            nc.scalar.copy(out, in_)
        else:
            nc.gpsimd.tensor_copy(out, in_)

    with tile.TileContext(nc) as tc:
        with (
            tc.tile_pool(name="consts", bufs=1) as consts,
            tc.tile_pool(name="xin", bufs=8) as xpool,
            tc.tile_pool(name="ysb", bufs=PIPE + 8) as ypool_sb,
            tc.tile_pool(name="osb", bufs=6) as opool_sb,
            tc.tile_pool(name="yps", bufs=2, space="PSUM") as ypool_ps,
            tc.tile_pool(name="ops", bufs=2, space="PSUM") as opool_ps,
        ):
            x_tiles = [None] * n_batches
            y_ps = [None] * GROUPS
            y_sb = [None] * GROUPS
            o_sb = [None] * GROUPS

            def dma_in(b, splits=(0, BATCH), pusher=None):
                if x_tiles[b] is None:
                    x_tiles[b] = xpool.tile([128, BATCH, 2, W], X_DT, name="x_sb")
                for lo, hi in zip(splits[:-1], splits[1:]):
                    (pusher or nc.sync).dma_start(
                        out=x_tiles[b][:, lo:hi],
                        in_=x_ext[:, b * BATCH + lo : b * BATCH + hi, :, :],
                    )

            # Band-matrix consts ride scalar's own HW DMA queue, in parallel
            # with the first x slivers on the sync queue: stage A has both
            # ~1us after the DGE rings spin up.  A 2-plane sliver leads so
            # the first group starts as soon as possible.
            bv_sb = consts.tile([128, 2, OUT_I], X_DT)
            nc.scalar.dma_start(
                out=bv_sb, in_=bv_ext[:, :].rearrange("(k r) o -> r k o", k=2)
            )
            bw_sb = consts.tile([128, 2, OUT_I], MM_DT)
            nc.scalar.dma_start(
                out=bw_sb, in_=bw_ext[:, :].rearrange("(k r) o -> r k o", k=2)
            )
            dma_in(0, splits=(0, 2, 8))
            warm_w = consts.tile([128, 128], X_DT)
            nc.gpsimd.memset(warm_w, 0.0)
            dma_in(0, splits=(8, 16))

            # Keep the PE HAM-warm while the first input DMA is in flight:
            # dummy matmuls on a memset scratch tile (no DMA dependency, so
            # they start the moment the runtime preamble ends).
            warm_ps = opool_ps.tile(
                [128, 2, 512], mybir.dt.float32, name="warm_ps", tag="o_ps"
            )
            for _ in range(N_WARM):
                nc.tensor.matmul(
                    warm_ps[:, 0, 0:128],
                    lhsT=warm_w,
                    rhs=warm_w,
                    start=True,
                    stop=True,
                )

            def banded_mms(out_col, lhsT_of_k, band_sb):
                """One output row-block: 3 matmuls streaming the nonzero
                128-col support of each contraction chunk of the band.
                out_col(c0, c1) -> PSUM slice for band columns [c0, c1)."""
                nc.tensor.matmul(
                    out_col(0, 128),
                    lhsT=lhsT_of_k(0),
                    rhs=band_sb[:, 0, 0:128],
                    start=True,
                    stop=False,
                )
                nc.tensor.matmul(
                    out_col(65, 128),
                    lhsT=lhsT_of_k(1),
                    rhs=band_sb[:, 1, 65:128],
                    start=False,
                    stop=True,
                )
                nc.tensor.matmul(
                    out_col(128, OUT_I),
                    lhsT=lhsT_of_k(1),
                    rhs=band_sb[:, 1, 128:OUT_I],
                    start=True,
                    stop=True,
                )

            def stage_a_group(g):
                b, p0 = divmod(2 * g, BATCH)
                if p0 == 0 and b > 0:
                    dma_in(b)
                x_sb = x_tiles[b]
                # 2-plane PSUM tile; each plane slot is one full 2 KiB bank
                y_ps[g] = ypool_ps.tile([128, 2, 512], mybir.dt.float32, name="y_ps")
                for s in range(2):
                    p = p0 + s
                    for m in range(2):  # w-chunk -> PSUM partitions
                        # DoubleRow: both 128-row k-tiles of h contract in one
                        # matmul; rhs [128, 2, 193] streams at 2 cols/cycle.
                        nc.tensor.matmul(
                            y_ps[g][:, s, m * OUT_I : (m + 1) * OUT_I],
                            lhsT=x_sb[:, p, :, m * 128 : (m + 1) * 128],
                            rhs=bv_sb,
                            start=True,
                            stop=True,
                            perf_mode=mybir.MatmulPerfMode.DoubleRow,
                        )

            OWID = OUT_I + 2 * M2  # 323 output columns per plane

            def stage_b_group(g):
                # evacuate stage A PSUM (both slots in one strided op) on the
                # statically assigned engine
                y_sb[g] = ypool_sb.tile([128, 2, 2 * OUT_I], MM_DT, name="y_sb")
                evac_copy(_EVAC_ENG[2 * g], y_sb[g], y_ps[g][:, :, 0 : 2 * OUT_I])
                y_ps[g] = None
                # [part, slot, 512]: slot = one 2 KiB bank holding the full
                # 323-col output of one plane: c0 [io, jo 0:193], then the
                # transposed c1 regions A [jo, io1] and B [jo1, io1]
                o_ps = opool_ps.tile(
                    [128, 2, 512], mybir.dt.float32, name="o_ps", tag="o_ps"
                )
                for s in range(2):
                    # c0: io chunk 0 on partitions, banded jo streaming
                    banded_mms(
                        lambda c0, c1, s=s: o_ps[0:128, s, c0:c1],
                        lambda k, s=s: y_sb[g][:, s, k * OUT_I : k * OUT_I + 128],
                        bw_sb,
                    )
                    # c1 transposed: jo on partitions, io 128..193 streamed
                    # (65 cols).  Region A (jo 0..128) accumulates both
                    # w-chunks as full M=128 matmuls -- each chunk's
                    # out-of-support jo columns are all-zero weight columns,
                    # which cost nothing.  Region B (jo 128..193) is w-chunk
                    # 1 alone.
                    rhs_k = lambda k, s=s: y_sb[g][
                        :, s, k * OUT_I + 128 : k * OUT_I + OUT_I
                    ]
                    nc.tensor.matmul(
                        o_ps[0:128, s, OUT_I : OUT_I + M2],
                        lhsT=bw_sb[:, 0, 0:128],
                        rhs=rhs_k(0),
                        start=True,
                        stop=False,
                    )
                    nc.tensor.matmul(
                        o_ps[0:128, s, OUT_I : OUT_I + M2],
                        lhsT=bw_sb[:, 1, 0:128],
                        rhs=rhs_k(1),
                        start=False,
                        stop=True,
                    )
                    nc.tensor.matmul(
                        o_ps[0:M2, s, OUT_I + M2 : OUT_I + 2 * M2],
                        lhsT=bw_sb[:, 1, 128:OUT_I],
                        rhs=rhs_k(1),
                        start=True,
                        stop=True,
                    )
                # o_sb mirrors o_ps's per-plane 323-col layout, plane-major,
                # so the evac is a straight dense copy and each flush reads
                # one long run per partition.
                q, r = divmod(g, FLUSH)
                if r == 0:
                    o_sb[q] = opool_sb.tile(
                        [128, 2 * FLUSH, OWID], OUT_DT, name="o_sb"
                    )
                evac_copy(
                    _EVAC_ENG[2 * g + 1],
                    o_sb[q][:, 2 * r : 2 * r + 2, :],
                    o_ps[:, :, 0:OWID],
                )
                # Flush FLUSH groups (16 planes) per tile from the otherwise
                # idle gpsimd engine (GPSIMD cannot touch PSUM, so descriptor
                # pushes are all it does).  The final tile drains in fine
                # slices, alternating onto the by-then idle sync input queue,
                # so the tail rides two queues and the last slice is tiny.
                last_tile = q == GROUPS // FLUSH - 1
                flush_points = (
                    {1: (0, 4, "y"), 3: (4, 8, "g"), 5: (8, 12, "g"),
                     6: (12, 14, "y"), 7: (14, 16, "y")}
                    if last_tile
                    # halves, so the output queue drains steadily instead of
                    # in late 1.3 MB lumps
                    else {FLUSH // 2 - 1: (0, FLUSH, "g"),
                          FLUSH - 1: (FLUSH, 2 * FLUSH, "g")}
                )
                if r in flush_points:
                    lo, hi, eng = flush_points[r]
                    p0 = 2 * (g - r)
                    pushers = {"g": nc.gpsimd, "y": nc.sync}
                    pushers[eng].dma_start(
                        out=outm_ext[:, p0 + lo : p0 + hi, :],
                        in_=o_sb[q][:, lo:hi, :],
                    )
                    if r == FLUSH - 1:
                        o_sb[q] = None

            for g in range(GROUPS + PIPE):
                if g < GROUPS:
                    stage_a_group(g)
                if g >= PIPE:
                    stage_b_group(g - PIPE)

    if split_waits:
        _split_multiwaits(nc)
    return nc


_NC_CACHE = None


def _get_nc():
    global _NC_CACHE
    if _NC_CACHE is None:
        _NC_CACHE = _build()
    return _NC_CACHE


def _run(x: np.ndarray, trace: bool = False):
    x = np.asarray(x, dtype=np.float32)
    assert x.shape == (N, C, H, W), x.shape
    xq = _diffuse_fp8(x)
    # partition-major repack: [core, plane, (k r), w] -> [core, r, plane, k, w]
    xs = xq.reshape(N_CORES, PLANES_PER_CORE, 2, 128, W).transpose(0, 3, 1, 2, 4)
    xs = np.ascontiguousarray(xs)
    bv = _band(H, KPOOL, 1.0).astype(X_NP)
    bw = _band(W, KPOOL, 1.0 / (KPOOL * KPOOL)).astype(MM_NP)
    in_maps = [{"x": xs[i], "bv": bv, "bw": bw} for i in range(N_CORES)]
    # The device sporadically reports NRT_EXEC_UNIT_UNRECOVERABLE even for a
    # known-good NEFF; retry a couple of times before giving up.
    last_err = None
    for attempt in range(3):
        try:
            res = run_bass_kernel_spmd(
                nc=_get_nc(),
                in_maps=in_maps,
                core_ids=list(range(N_CORES)),
                trace=trace,
            )
            break
        except Exception as e:  # noqa: BLE001
            last_err = e
            import time

            time.sleep(2.0 * (attempt + 1))
    else:
        raise last_err
    # unpack outm [128, plane, 323]: c0 [io, plane, jo]; A [jo, plane, io1];
    # B [jo1, plane, io1] (only partitions 0:65 of B are live)
    outs = []
    for i in range(N_CORES):
        m = np.asarray(res.results[i]["outm"], dtype=np.float32)
        inner = np.empty((PLANES_PER_CORE, OUT_I, OUT_I), dtype=np.float32)
        inner[:, 0:128, :] = m[:, :, 0:OUT_I].transpose(1, 0, 2)
        inner[:, 128:OUT_I, 0:128] = m[:, :, OUT_I : OUT_I + M2].transpose(1, 2, 0)
        inner[:, 128:OUT_I, 128:OUT_I] = m[
            0:M2, :, OUT_I + M2 : OUT_I + 2 * M2
        ].transpose(1, 2, 0)
        outs.append(inner)
    inner = np.stack(outs, axis=0)  # [cores, planes, 193, 193]
    full = np.pad(
        inner, ((0, 0), (0, 0), (PAD_LO, PAD_HI), (PAD_LO, PAD_HI)), mode="edge"
    )
    return full.reshape(N, C, H, W), res


def kernel(x: np.ndarray) -> np.ndarray:
    out, _ = _run(x, trace=False)
    return out
